# revision 1
# baseline (speedup 1.0000x reference)
"""Trainium2 Bass kernel for nn_AttnBlock (B=16, C=512, H=W=32, T=180, G=32).

Math: the module broadcasts the text condition across channels, so k/v rows are
identical for every channel and the whole attention block collapses to rank-1:

  per batch b:
    group-norm stats over x[b]:   mu_g, rstd_g (32 groups of 16 ch x 1024 pix)
    wq_colsum[c] = sum_o wq[o,c];  a[c] = wq_colsum[c]*gamma[c]*rstd_{g(c)}
    s[n]   = sum_c a[c]*x[c,n] + const_b           (const_b folds mu/beta/bq)
    kb[f]  = wk @ cond_b + bk ;  vb[f] = wv @ cond_b + bv
    e[f,n] = exp(SCALE * kb[f] * s[n])
    w[n]   = (sum_f vb[f]*e[f,n]) / (sum_f e[f,n])
    out[c,n] = x[c,n] + wo_rowsum[c]*w[n] + bo[c]

Sharding: data-parallel over batch, 2 batches per core, 8 cores, no collectives.
PSUM: per (batch, half) one packed [128,512] accumulator bank holds the s-matvec
row at partition 0, the vb-weighted softmax numerator at partition 32, and the
softmax denominator at partition 64 (legal engine AP starts are 0/32/64/96).
"""
import numpy as np
from contextlib import ExitStack

B, C, HW, N, T = 16, 512, 32, 1024, 180
F = 1024                      # in_features == H*W
G = 32                        # groups; 16 channels per group
NCORES, BPC = 8, 2            # cores, batches per core
NCH = C // 128                # 4 channel chunks
NFC = F // 128                # 8 feature chunks
EPS = 1e-6
SCALE = float(C) ** -0.5

_CACHE = {}


def _legalize_sync(nc, mybir):
    """This walrus build accepts at most one sync-wait command per
    instruction; hoist extra waits onto preceding same-engine NOPs."""
    k = 0
    for fn in nc.m.functions:
        for blk in fn.blocks:
            new = []
            for ins in blk.instructions:
                si = ins.sync_info
                if si is not None and si.on_wait is not None and len(si.on_wait) > 1:
                    for w in list(si.on_wait[:-1]):
                        nop = mybir.InstNoOp(name=f"syncsplit-{k}", ins=[], outs=[])
                        k += 1
                        nop.engine = ins.engine
                        nop.sync_info = mybir.SyncInfo(on_wait=[w], on_update=[])
                        new.append(nop)
                    ins.sync_info = mybir.SyncInfo(
                        on_wait=[si.on_wait[-1]],
                        on_update=list(si.on_update or []))
                new.append(ins)
            blk.instructions[:] = new


def _build(reps=1, legalize=True):
    import concourse.bass as bass
    import concourse.mybir as mybir
    import concourse.tile as tile
    from concourse.tile import add_dep_helper

    f32 = mybir.dt.float32
    bf16 = mybir.dt.bfloat16
    Act = mybir.ActivationFunctionType
    Alu = mybir.AluOpType

    nc = bass.Bass()

    x_d = nc.dram_tensor("x_sh", [BPC, C, N], f32, kind="ExternalInput")
    cond_d = nc.dram_tensor("cond_sh", [BPC, T], f32, kind="ExternalInput")
    gamma_d = nc.dram_tensor("gamma", [C], f32, kind="ExternalInput")
    beta_d = nc.dram_tensor("beta", [C], f32, kind="ExternalInput")
    wq_d = nc.dram_tensor("wq", [C, C], f32, kind="ExternalInput")
    bq_d = nc.dram_tensor("bq", [C], f32, kind="ExternalInput")
    wk_d = nc.dram_tensor("wk", [F, T], f32, kind="ExternalInput")
    bk_d = nc.dram_tensor("bk", [F], f32, kind="ExternalInput")
    wv_d = nc.dram_tensor("wv", [F, T], f32, kind="ExternalInput")
    bv_d = nc.dram_tensor("bv", [F], f32, kind="ExternalInput")
    wo_d = nc.dram_tensor("wo", [C, C], f32, kind="ExternalInput")
    bo_d = nc.dram_tensor("bo", [C], f32, kind="ExternalInput")
    ind128_d = nc.dram_tensor("ind128", [128, 8], f32, kind="ExternalInput")
    indT8_d = nc.dram_tensor("indT8", [8, 128], f32, kind="ExternalInput")
    out_d = nc.dram_tensor("out", [BPC, C, N], f32, kind="ExternalOutput")

    with tile.TileContext(nc) as tc, ExitStack() as ctx:
        singles = ctx.enter_context(tc.tile_pool(name="singles", bufs=1))
        wtmp = ctx.enter_context(tc.tile_pool(name="wtmp", bufs=1))
        xpool = ctx.enter_context(tc.tile_pool(name="xpool", bufs=2))
        xbpool = ctx.enter_context(tc.tile_pool(name="xbpool", bufs=2))
        epool = ctx.enter_context(tc.tile_pool(name="epool", bufs=8))
        ypool = ctx.enter_context(tc.tile_pool(name="ypool", bufs=4))
        opool = ctx.enter_context(tc.tile_pool(name="opool", bufs=4))
        bpool = ctx.enter_context(tc.tile_pool(name="bpool", bufs=2))
        ps_tiny = ctx.enter_context(tc.tile_pool(name="ps_tiny", bufs=2, space="PSUM"))
        ps_acc = ctx.enter_context(tc.tile_pool(name="ps_acc", bufs=4, space="PSUM"))
        ps_rep = ctx.enter_context(tc.tile_pool(name="ps_rep", bufs=2, space="PSUM"))

        # constants + ACT table preload first (ACT ring is in-order)
        ones_col = singles.tile([128, 1], f32)
        nc.vector.memset(ones_col, 1.0)
        ones_col_b = singles.tile([128, 1], bf16)
        nc.vector.memset(ones_col_b, 1.0)
        ones_row_b = singles.tile([1, 128], bf16)
        nc.vector.memset(ones_row_b, 1.0)
        eps8 = singles.tile([8, 1], f32)
        nc.vector.memset(eps8, EPS)
        tl = singles.tile([1, 1], f32)
        nc.scalar.activation(tl, eps8[0:1, 0:1], Act.Exp)  # preload exp table

        # ---------------- prologue: loads in dependency-priority order ----------
        xts, conds = [], []
        # weights ride the second HWDGE ring (ACT sequencer), x rides SP
        wq_all = wtmp.tile([128, NCH, C], f32, tag="wq")
        nc.scalar.dma_start(wq_all, wq_d[:, :].rearrange("(a p) c -> p a c", p=128))
        xt0 = xpool.tile([128, NCH, N], f32, tag="xt", name="xt0")
        for ch in range(NCH):
            eng = nc.sync if ch < 3 else nc.gpsimd
            for hh in range(2):
                eng.dma_start(
                    xt0[:, ch, 512 * hh:512 * (hh + 1)],
                    x_d[0, 128 * ch:128 * (ch + 1), 512 * hh:512 * (hh + 1)])
        xts.append(xt0)
        xt1 = xpool.tile([128, NCH, N], f32, tag="xt", name="xt1")
        for ch in range(NCH):
            nc.sync.dma_start(xt1[:, ch, :], x_d[1, 128 * ch:128 * (ch + 1), :])
        xts.append(xt1)
        for b in range(BPC):
            cond_rep = bpool.tile([128, T], f32, tag="cond", name=f"cond{b}")
            nc.scalar.dma_start(cond_rep, cond_d[b:b + 1, :].to_broadcast([128, T]))
            conds.append(cond_rep)
        ind128 = singles.tile([128, 8], f32)
        nc.scalar.dma_start(ind128, ind128_d[:, :])
        indT8 = singles.tile([8, 128], f32)
        nc.scalar.dma_start(indT8, indT8_d[:, :])
        gamma_pc = singles.tile([128, NCH], f32)
        nc.scalar.dma_start(gamma_pc, gamma_d[:].rearrange("(a p) -> p a", p=128))
        beta_pc = singles.tile([128, NCH], f32)
        nc.scalar.dma_start(beta_pc, beta_d[:].rearrange("(a p) -> p a", p=128))
        bq_pc = singles.tile([128, NCH], f32)
        nc.scalar.dma_start(bq_pc, bq_d[:].rearrange("(a p) -> p a", p=128))
        wkv = singles.tile([128, 2 * NFC, T], f32)
        nc.gpsimd.dma_start(wkv[:, 0:NFC, :],
                            wk_d[:, :].rearrange("(a p) t -> p a t", p=128))
        nc.gpsimd.dma_start(wkv[:, NFC:2 * NFC, :],
                            wv_d[:, :].rearrange("(a p) t -> p a t", p=128))

        # ---------------- setup: remaining small layouts ----------------
        bk_pc = singles.tile([128, NFC], f32)
        nc.gpsimd.dma_start(bk_pc, bk_d[:].rearrange("(a p) -> p a", p=128))
        bv_pc = singles.tile([128, NFC], f32)
        nc.gpsimd.dma_start(bv_pc, bv_d[:].rearrange("(a p) -> p a", p=128))
        bks_pc = singles.tile([128, NFC], f32)
        nc.vector.tensor_scalar_mul(bks_pc, bk_pc, SCALE)
        bo_pc = singles.tile([128, NCH], f32)
        nc.gpsimd.dma_start(bo_pc, bo_d[:].rearrange("(a p) -> p a", p=128))

        # wq colsum[c] = sum_o wq[o,c], via PE: 16 tiny matmuls accumulate over o-chunks
        colsum_pc = singles.tile([128, NCH], f32)
        for cj in range(NCH):
            cs_ps = ps_tiny.tile([128, 1], f32, tag="tiny")
            for oc in range(NCH):
                nc.tensor.matmul(
                    cs_ps, wq_all[:, oc, 128 * cj:128 * (cj + 1)], ones_col,
                    start=(oc == 0), stop=(oc == NCH - 1))
            nc.vector.tensor_copy(colsum_pc[:, cj:cj + 1], cs_ps)

        wo_sum = singles.tile([128, NCH], f32)

        def emit_wo_sum():
            wo_all = wtmp.tile([128, NCH, C], f32, tag="wo")
            nc.sync.dma_start(wo_all,
                              wo_d[:, :].rearrange("(a p) c -> p a c", p=128))
            nc.vector.tensor_reduce(wo_sum, wo_all, axis=mybir.AxisListType.X,
                                    op=Alu.add)

        wg = singles.tile([128, NCH], f32)
        nc.vector.tensor_mul(wg, colsum_pc, gamma_pc)
        cbeta = singles.tile([128, NCH], f32)
        nc.vector.tensor_mul(cbeta, colsum_pc, beta_pc)

        # bqwcb = sum(bq) + sum_c colsum*beta  (scalar in [1,1])
        bqwcb_ps = ps_tiny.tile([1, 1], f32, tag="tiny")
        for ci in range(NCH):
            nc.tensor.matmul(bqwcb_ps, cbeta[:, ci:ci + 1], ones_col,
                             start=(ci == 0), stop=False)
        for ci in range(NCH):
            nc.tensor.matmul(bqwcb_ps, bq_pc[:, ci:ci + 1], ones_col,
                             start=False, stop=(ci == NCH - 1))
        bqwcb = singles.tile([1, 1], f32)
        nc.vector.tensor_copy(bqwcb, bqwcb_ps)

        # ---------------- per-batch stages (software-pipelined emission) ----
        S = [dict() for _ in range(BPC)]

        def stage_load(b, rep_i):
            if rep_i == 0:
                S[b]["xt"] = xts[b]
                S[b]["cond"] = conds[b]
            else:
                xt = xpool.tile([128, NCH, N], f32, tag="xt", name=f"xtr{b}")
                for ch in range(NCH):
                    nc.sync.dma_start(xt[:, ch, :],
                                      x_d[b, 128 * ch:128 * (ch + 1), :])
                cond_rep = bpool.tile([128, T], f32, tag="cond", name=f"condr{b}")
                nc.sync.dma_start(cond_rep,
                                  cond_d[b:b + 1, :].to_broadcast([128, T]))
                S[b]["xt"] = xt
                S[b]["cond"] = cond_rep

        def stage_cast(b):
            xb = xbpool.tile([128, NCH, N], bf16, tag="xb", name=f"xb{b}")
            ci = nc.gpsimd.tensor_copy(xb, S[b]["xt"])
            S[b]["xb"] = xb
            S[b]["cast_inst"] = ci

        def stage_stats(b):
            xt = S[b]["xt"]
            mv2 = bpool.tile([128, NCH, 2], f32, tag="mv2", name=f"mv2_{b}")
            mv = bpool.tile([128, NCH, 2], f32, tag="mv", name=f"mv_{b}")
            for ch in range(NCH):
                st = bpool.tile([128, 2, 6], f32, tag="st", name=f"st{b}{ch}")
                nc.vector.bn_stats(st[:, 0, :], xt[:, ch, 0:512])
                nc.vector.bn_stats(st[:, 1, :], xt[:, ch, 512:1024])
                nc.vector.bn_aggr(mv[:, ch, :], st)
            msq = bpool.tile([128, NCH], f32, tag="msq", name=f"msq{b}")
            nc.vector.tensor_mul(msq, mv[:, :, 0], mv[:, :, 0])
            nc.vector.tensor_copy(mv2[:, :, 0], mv[:, :, 0])
            nc.vector.tensor_add(mv2[:, :, 1], mv[:, :, 1], msq)
            gstat_ps = ps_tiny.tile([8, NCH, 2], f32, tag="tiny", name=f"gst{b}")
            for ch in range(NCH):
                nc.tensor.matmul(gstat_ps[:, ch, :], ind128, mv2[:, ch, :],
                                 start=True, stop=True)
            gsb = bpool.tile([8, NCH, 2], f32, tag="gsb", name=f"gsb{b}")
            nc.scalar.copy(gsb, gstat_ps)
            msqg = bpool.tile([8, NCH], f32, tag="msqg", name=f"msqg{b}")
            nc.vector.tensor_mul(msqg, gsb[:, :, 0], gsb[:, :, 0])
            varg = bpool.tile([8, NCH], f32, tag="varg", name=f"varg{b}")
            nc.vector.tensor_sub(varg, gsb[:, :, 1], msqg)
            lnv = bpool.tile([8, NCH], f32, tag="lnv", name=f"lnv{b}")
            nc.scalar.activation(lnv, varg, Act.Ln, bias=eps8[:, 0:1])
            rm = bpool.tile([8, 2, NCH], f32, tag="rm", name=f"rm{b}")
            nc.scalar.activation(rm[:, 0, :], lnv, Act.Exp, scale=-0.5)
            nc.vector.tensor_mul(rm[:, 1, :], gsb[:, :, 0], rm[:, 0, :])
            rep_ps = ps_rep.tile([128, 2 * NCH], f32, tag="rep", name=f"rep{b}")
            nc.tensor.matmul(rep_ps, indT8, rm.rearrange("g a c -> g (a c)"),
                             start=True, stop=True)
            rep3 = rep_ps.rearrange("p (a c) -> p a c", a=2)
            a_all = bpool.tile([128, NCH], bf16, tag="a_all", name=f"a_all{b}")
            nc.vector.tensor_mul(a_all, wg, rep3[:, 0, :])
            wm_all = bpool.tile([128, NCH], f32, tag="wm_all", name=f"wm{b}")
            nc.vector.tensor_mul(wm_all, wg, rep3[:, 1, :])
            S[b]["a_all"], S[b]["wm_all"] = a_all, wm_all

        def stage_kv(b):
            cond_rep = S[b]["cond"]
            cond_b8 = bass.AP(
                tensor=cond_rep.tensor, offset=cond_rep.offset,
                ap=[list(cond_rep.ap[0]), [0, NFC], list(cond_rep.ap[1])])
            kjunk = bpool.tile([128, NFC, T], f32, tag="kjunk", name=f"kj{b}")
            ki = nc.gpsimd.tensor_tensor(kjunk, wkv[:, 0:NFC, :], cond_b8, Alu.mult)
            if b == 0 and "cast_inst" in S[b]:
                add_dep_helper(ki.ins, S[b]["cast_inst"].ins, sync=False,
                               reason="keep pool cast ahead of kv mult")
            kt1 = bpool.tile([128, NFC, 90], f32, tag="kt1", name=f"kt1{b}")
            nc.gpsimd.tensor_add(kt1, kjunk[:, :, 0:90], kjunk[:, :, 90:180])
            kt2 = bpool.tile([128, NFC, 45], f32, tag="kt2", name=f"kt2{b}")
            nc.gpsimd.tensor_add(kt2, kt1[:, :, 0:45], kt1[:, :, 45:90])
            kraw = bpool.tile([128, NFC], f32, tag="kraw", name=f"kraw{b}")
            nc.vector.tensor_reduce(kraw, kt2, axis=mybir.AxisListType.X,
                                    op=Alu.add)
            kbs = bpool.tile([128, NFC], f32, tag="kbs", name=f"kbs{b}")
            nc.vector.tensor_scalar_mul(kbs, kraw, SCALE)
            nc.vector.tensor_add(kbs, kbs, bks_pc)
            vjunk = bpool.tile([128, NFC, T], f32, tag="vjunk", name=f"vj{b}")
            nc.gpsimd.tensor_tensor(vjunk, wkv[:, NFC:2 * NFC, :], cond_b8,
                                    Alu.mult)
            vt1 = bpool.tile([128, NFC, 90], f32, tag="vt1", name=f"vt1{b}")
            nc.gpsimd.tensor_add(vt1, vjunk[:, :, 0:90], vjunk[:, :, 90:180])
            vt2 = bpool.tile([128, NFC, 45], f32, tag="vt2", name=f"vt2{b}")
            nc.gpsimd.tensor_add(vt2, vt1[:, :, 0:45], vt1[:, :, 45:90])
            vraw = bpool.tile([128, NFC], f32, tag="vraw", name=f"vraw{b}")
            nc.vector.tensor_reduce(vraw, vt2, axis=mybir.AxisListType.X,
                                    op=Alu.add)
            vbp_b = bpool.tile([128, NFC], bf16, tag="vbp_b", name=f"vbp{b}")
            nc.vector.tensor_add(vbp_b, vraw, bv_pc)
            # [vb | zeros*31 | ones] per fc: one M=33 matmul yields num@p32, Z@p64
            vbones = bpool.tile([128, NFC, 33], bf16, tag="vbones", name=f"vo{b}")
            nc.gpsimd.memset(vbones, 0.0)
            nc.gpsimd.tensor_copy(vbones[:, :, 0:1],
                                  vbp_b.rearrange("p (f o) -> p f o", o=1))
            nc.gpsimd.memset(vbones[:, :, 32:33], 1.0)
            S[b]["kbs"], S[b]["vbones"] = kbs, vbones

        def stage_smv_mm(b):
            a_all, wm_all, xb = S[b]["a_all"], S[b]["wm_all"], S[b]["xb"]
            acc = [ps_acc.tile([128, 512], f32, tag="acc", name=f"acc{b}{h}")
                   for h in range(2)]
            wms_ps = ps_tiny.tile([1, 1], f32, tag="tiny", name=f"wms{b}")
            for ch in range(NCH):
                for h in range(2):
                    nc.tensor.matmul(
                        acc[h][0:1, :], a_all[:, ch:ch + 1],
                        xb[:, ch, 512 * h:512 * (h + 1)],
                        start=(ch == 0), stop=(ch == NCH - 1),
                        skip_group_check=True)
                nc.tensor.matmul(wms_ps, wm_all[:, ch:ch + 1], ones_col,
                                 start=(ch == 0), stop=(ch == NCH - 1))
            S[b]["acc"], S[b]["wms_ps"] = acc, wms_ps

        def stage_s(b):
            acc, wms_ps = S[b]["acc"], S[b]["wms_ps"]
            constb = bpool.tile([1, 1], f32, tag="constb", name=f"cb{b}")
            nc.vector.tensor_sub(constb, bqwcb, wms_ps)
            s_sb = bpool.tile([1, N], bf16, tag="s_sb", name=f"s_sb{b}")
            for h in range(2):
                if b == 0:
                    nc.scalar.activation(s_sb[0:1, 512 * h:512 * (h + 1)],
                                         acc[h][0:1, :], Act.Identity,
                                         bias=constb[0:1, 0:1])
                else:
                    nc.vector.tensor_scalar_add(
                        s_sb[0:1, 512 * h:512 * (h + 1)],
                        acc[h][0:1, :], constb[0:1, 0:1])
            srep_sb = bpool.tile([128, N], bf16, tag="srep_sb", name=f"srep{b}")
            for h in range(2):
                srep_ps = ps_rep.tile([128, 512], f32, tag="rep",
                                      name=f"srep{b}{h}")
                nc.tensor.matmul(srep_ps, ones_row_b,
                                 s_sb[0:1, 512 * h:512 * (h + 1)],
                                 start=True, stop=True)
                nc.scalar.copy(srep_sb[:, 512 * h:512 * (h + 1)], srep_ps)
            S[b]["srep_sb"] = srep_sb

        def stage_expz(b, mid=None):
            acc, srep_sb = S[b]["acc"], S[b]["srep_sb"]
            kbs, vbones = S[b]["kbs"], S[b]["vbones"]
            for fc in range(NFC):
                if fc == 3 and mid is not None:
                    mid()
                e_sb = epool.tile([128, N], bf16, tag="e", name=f"e{b}{fc}")
                if fc < NFC - 1:
                    nc.scalar.activation(e_sb, srep_sb, Act.Exp,
                                         scale=kbs[:, fc:fc + 1])
                    for h in range(2):
                        eh = e_sb[:, 512 * h:512 * (h + 1)]
                        nc.tensor.matmul(acc[h][0:33, :], vbones[:, fc, :], eh,
                                         start=(fc == 0), stop=False,
                                         skip_group_check=True)
                else:
                    # split the final fc so h0's accumulation (and the w-stage
                    # reciprocal chain) can start while h1's exp still runs
                    for h in range(2):
                        eh = e_sb[:, 512 * h:512 * (h + 1)]
                        nc.scalar.activation(eh,
                                             srep_sb[:, 512 * h:512 * (h + 1)],
                                             Act.Exp, scale=kbs[:, fc:fc + 1])
                        nc.tensor.matmul(acc[h][0:33, :], vbones[:, fc, :], eh,
                                         start=False, stop=True,
                                         skip_group_check=True)

        def stage_w(b):
            acc = S[b]["acc"]
            w_sb = bpool.tile([1, N], bf16, tag="w_sb", name=f"w_sb{b}")
            for h in range(2):
                zr = bpool.tile([1, 512], f32, tag="zr", name=f"zr{b}{h}")
                nc.vector.reciprocal(zr, acc[h][32:33, :])
                nc.vector.tensor_mul(w_sb[0:1, 512 * h:512 * (h + 1)],
                                     acc[h][0:1, :], zr)
            wrep_sb = bpool.tile([128, N], bf16, tag="wrep_sb", name=f"wrep{b}")
            for h in range(2):
                wrep_ps = ps_rep.tile([128, 512], f32, tag="rep",
                                      name=f"wrep{b}{h}")
                nc.tensor.matmul(wrep_ps, ones_row_b,
                                 w_sb[0:1, 512 * h:512 * (h + 1)],
                                 start=True, stop=True)
                if b == 1:
                    nc.scalar.copy(wrep_sb[:, 512 * h:512 * (h + 1)], wrep_ps)
                else:
                    nc.vector.tensor_copy(wrep_sb[:, 512 * h:512 * (h + 1)],
                                          wrep_ps)
            S[b]["wrep_sb"] = wrep_sb

        def stage_yout(b):
            xt, wrep_sb = S[b]["xt"], S[b]["wrep_sb"]
            for ch in range(NCH):
                y_sb = ypool.tile([128, N], bf16, tag="y", name=f"y{b}{ch}")
                if b == 1 and ch >= 2:
                    nc.scalar.activation(y_sb, wrep_sb, Act.Identity,
                                         scale=wo_sum[:, ch:ch + 1],
                                         bias=bo_pc[:, ch:ch + 1])
                else:
                    nc.vector.tensor_scalar(y_sb, wrep_sb, wo_sum[:, ch:ch + 1],
                                            bo_pc[:, ch:ch + 1], op0=Alu.mult,
                                            op1=Alu.add)
                o_sb = opool.tile([128, N], f32, tag="o", name=f"o{b}{ch}")
                if b == 1 and ch >= 2:
                    nc.vector.tensor_add(o_sb, xt[:, ch, :], y_sb)
                else:
                    nc.gpsimd.tensor_add(o_sb, xt[:, ch, :], y_sb)
                eng = nc.scalar if (b == 1 and ch >= 2) else nc.sync
                eng.dma_start(out_d[b, 128 * ch:128 * (ch + 1), :], o_sb)

        for rep_i in range(reps):
            stage_load(0, rep_i)
            stage_load(1, rep_i)
            stage_cast(0)
            stage_stats(0)
            stage_smv_mm(0)
            stage_s(0)
            stage_kv(0)

            def _mid():
                stage_cast(1)
                stage_stats(1)
                stage_smv_mm(1)
                if rep_i == 0:
                    emit_wo_sum()

            stage_expz(0, mid=_mid)
            stage_s(1)
            stage_kv(1)
            stage_w(0)
            stage_expz(1)
            stage_yout(0)
            stage_w(1)
            stage_yout(1)

    if legalize:
        _legalize_sync(nc, mybir)
    return nc


def _indicators():
    ind128 = np.zeros((128, 8), np.float32)
    indT8 = np.zeros((8, 128), np.float32)
    for g in range(8):
        ind128[16 * g:16 * g + 16, g] = 1.0 / 16.0
        indT8[g, 16 * g:16 * g + 16] = 1.0
    return ind128, indT8


def kernel(**inputs):
    from concourse.bass_utils import run_bass_kernel_spmd

    if "nc" not in _CACHE:
        _CACHE["nc"] = _build()
    nc = _CACHE["nc"]

    f = {k: np.ascontiguousarray(np.asarray(v, dtype=np.float32))
         for k, v in inputs.items()}
    x = f["x"].reshape(B, C, N)
    cond = f["condition"]
    ind128, indT8 = _indicators()

    in_maps = []
    for i in range(NCORES):
        in_maps.append({
            "x_sh": np.ascontiguousarray(x[BPC * i:BPC * (i + 1)]),
            "cond_sh": np.ascontiguousarray(cond[BPC * i:BPC * (i + 1)]),
            "gamma": f["gamma"], "beta": f["beta"],
            "wq": f["wq"], "bq": f["bq"],
            "wk": f["wk"], "bk": f["bk"],
            "wv": f["wv"], "bv": f["bv"],
            "wo": f["wo"], "bo": f["bo"],
            "ind128": ind128, "indT8": indT8,
        })

    res = run_bass_kernel_spmd(nc, in_maps, core_ids=list(range(NCORES)))
    _CACHE["last_results"] = res
    out = np.concatenate([r["out"] for r in res.results], axis=0)
    return out.reshape(B, C, HW, HW).astype(np.float32)



# revision 17
# speedup vs baseline: 2.4611x; 2.4611x over previous
"""Trainium2 Bass kernel for nn_AttnBlock (B=16, C=512, H=W=32, T=180, G=32).

Math: the module broadcasts the text condition across channels, so k/v rows are
identical for every channel and the whole attention block collapses to rank-1:

  per batch b:
    group-norm stats over x[b]:   mu_g, rstd_g (32 groups of 16 ch x 1024 pix)
    wqg[c] = (sum_o wq[o,c]) * gamma[c];  a[c] = wqg[c]*rstd_{g(c)}
    s[n]   = sum_c a[c]*x[c,n]            (+ const_b, folded into the exp bias)
    kb[f]  = SCALE*(wk @ cond_b + bk) ;  vb[f] = wv @ cond_b + bv
    e[f,n] = exp(kb[f]*s[n] + kb[f]*const_b)
    w[n]   = (sum_f vb[f]*e[f,n]) / (sum_f e[f,n])
    out[c,n] = x[c,n] + wo_rowsum[c]*w[n] + bo[c]

Sharding: data-parallel over batch, 2 batches per core, 8 cores, no collectives.

v2: x/out/cond ride HBM as bf16 (halves DMA, kills the cast, 2x DVE modes);
k/v matvecs run on PE against host-transposed bf16 weights; exp reads the
s-broadcast straight from PSUM with const_b folded into a per-partition bias;
weight-derived scalars (wq colsum, wo rowsum, bq/beta folds) are host-folded.
PSUM: 4 accumulator banks (2/batch: s row @p0, softmax num @p0/den @p32 after
re-start) + 2 broadcast banks (srep/wrep, reused h0->h1) + 2 small banks.
"""
import numpy as np
from contextlib import ExitStack

B, C, HW, N, T = 16, 512, 32, 1024, 180
F = 1024                      # in_features == H*W
G = 32                        # groups; 16 channels per group
NCORES, BPC = 8, 2            # cores, batches per core
NCH = C // 128                # 4 channel chunks
NFC = F // 128                # 8 feature chunks
EPS = 1e-6
SCALE = float(C) ** -0.5

_CACHE = {}


def _legalize_sync(nc, mybir):
    """This walrus build accepts at most one sync-wait command per
    instruction; hoist extra waits onto preceding same-engine NOPs."""
    k = 0
    for fn in nc.m.functions:
        for blk in fn.blocks:
            new = []
            for ins in blk.instructions:
                si = ins.sync_info
                if si is not None and si.on_wait is not None and len(si.on_wait) > 1:
                    for w in list(si.on_wait[:-1]):
                        nop = mybir.InstNoOp(name=f"syncsplit-{k}", ins=[], outs=[])
                        k += 1
                        nop.engine = ins.engine
                        nop.sync_info = mybir.SyncInfo(on_wait=[w], on_update=[])
                        new.append(nop)
                    ins.sync_info = mybir.SyncInfo(
                        on_wait=[si.on_wait[-1]],
                        on_update=list(si.on_update or []))
                new.append(ins)
            blk.instructions[:] = new


def _build(reps=1, legalize=True, store_out=True, load_x=True, xbufs=4):
    import concourse.bass as bass
    import concourse.mybir as mybir
    import concourse.tile as tile

    f32 = mybir.dt.float32
    bf16 = mybir.dt.bfloat16
    Act = mybir.ActivationFunctionType
    Alu = mybir.AluOpType

    nc = bass.Bass()

    x_d = nc.dram_tensor("x_sh", [BPC, C, N], bf16, kind="ExternalInput")
    cond_d = nc.dram_tensor("cond_sh", [BPC, T], bf16, kind="ExternalInput")
    wqg_d = nc.dram_tensor("wqg", [C], f32, kind="ExternalInput")
    wors_d = nc.dram_tensor("wo_rs", [C], f32, kind="ExternalInput")
    bo_d = nc.dram_tensor("bo", [C], f32, kind="ExternalInput")
    bks_d = nc.dram_tensor("bks", [F], f32, kind="ExternalInput")
    bv_d = nc.dram_tensor("bv", [F], f32, kind="ExternalInput")
    wqb_d = nc.dram_tensor("wqb", [1], f32, kind="ExternalInput")
    wkT_d = nc.dram_tensor("wkT", [T, F], bf16, kind="ExternalInput")
    wvT_d = nc.dram_tensor("wvT", [T, F], bf16, kind="ExternalInput")
    ind128_d = nc.dram_tensor("ind128", [128, 8], f32, kind="ExternalInput")
    indT8_d = nc.dram_tensor("indT8", [8, 128], f32, kind="ExternalInput")
    out_d = nc.dram_tensor("out", [BPC, C, N], bf16, kind="ExternalOutput")

    with tile.TileContext(nc) as tc, ExitStack() as ctx:
        singles = ctx.enter_context(tc.tile_pool(name="singles", bufs=1))
        xpool = ctx.enter_context(tc.tile_pool(name="xpool", bufs=xbufs))
        epool = ctx.enter_context(tc.tile_pool(name="epool", bufs=8))
        ypool = ctx.enter_context(tc.tile_pool(name="ypool", bufs=4))
        opool = ctx.enter_context(tc.tile_pool(name="opool", bufs=2))
        bpool = ctx.enter_context(tc.tile_pool(name="bpool", bufs=2))
        ps_acc = ctx.enter_context(tc.tile_pool(name="ps_acc", bufs=4, space="PSUM"))
        ps_srep = ctx.enter_context(tc.tile_pool(name="ps_srep", bufs=1, space="PSUM"))
        ps_sm = ctx.enter_context(tc.tile_pool(name="ps_sm", bufs=2, space="PSUM"))

        # constants + ACT exp-table preload first (ACT ring is in-order)
        ones_col = singles.tile([128, 1], f32)
        nc.vector.memset(ones_col, 1.0)
        ones_row_b = singles.tile([1, 128], bf16)
        nc.vector.memset(ones_row_b, 1.0)
        eps8 = singles.tile([8, 1], f32)
        nc.vector.memset(eps8, EPS)
        tl = singles.tile([1, 1], f32)
        nc.scalar.activation(tl, eps8[0:1, 0:1], Act.Exp)

        # ---------------- prologue loads (ACT ring; idle during body) -------
        xts, condTs = [], []
        xt0 = xpool.tile([128, NCH, N], bf16, tag="xt", name="xt0")
        nc.sync.dma_start(xt0, x_d[0].rearrange("(a p) n -> p a n", p=128))
        xt1 = xpool.tile([128, NCH, N], bf16, tag="xt", name="xt1")
        nc.sync.dma_start(xt1, x_d[1].rearrange("(a p) n -> p a n", p=128))
        xts.append(xt0)
        xts.append(xt1)
        for b in range(BPC):
            cta = bpool.tile([128, 1], bf16, tag="cta", name=f"cta{b}")
            nc.sync.dma_start(cta, cond_d[b, 0:128].rearrange("(p o) -> p o", o=1))
            ctb = bpool.tile([52, 1], bf16, tag="ctb", name=f"ctb{b}")
            nc.sync.dma_start(ctb, cond_d[b, 128:180].rearrange("(p o) -> p o", o=1))
            condTs.append((cta, ctb))
        wkTa = singles.tile([128, F], bf16)
        nc.scalar.dma_start(wkTa, wkT_d[0:128, :])
        wkTb = singles.tile([52, F], bf16)
        nc.scalar.dma_start(wkTb, wkT_d[128:180, :])
        wvTa = singles.tile([128, F], bf16)
        nc.scalar.dma_start(wvTa, wvT_d[0:128, :])
        wvTb = singles.tile([52, F], bf16)
        nc.scalar.dma_start(wvTb, wvT_d[128:180, :])
        ind128 = singles.tile([128, 8], f32)
        nc.scalar.dma_start(ind128, ind128_d[:, :])
        indT8 = singles.tile([8, 128], f32)
        nc.scalar.dma_start(indT8, indT8_d[:, :])
        wqg_pc = singles.tile([128, NCH], f32)
        nc.scalar.dma_start(wqg_pc, wqg_d[:].rearrange("(a p) -> p a", p=128))
        wors_pc = singles.tile([128, NCH], f32)
        nc.scalar.dma_start(wors_pc, wors_d[:].rearrange("(a p) -> p a", p=128))
        bo_pc = singles.tile([128, NCH], f32)
        nc.scalar.dma_start(bo_pc, bo_d[:].rearrange("(a p) -> p a", p=128))
        bks_pc = singles.tile([128, NFC], f32)
        nc.scalar.dma_start(bks_pc, bks_d[:].rearrange("(a p) -> p a", p=128))
        bv_pc = singles.tile([128, NFC], f32)
        nc.scalar.dma_start(bv_pc, bv_d[:].rearrange("(a p) -> p a", p=128))
        wqb_sb = singles.tile([1, 1], f32)
        nc.scalar.dma_start(wqb_sb, wqb_d[:].rearrange("(p o) -> p o", o=1))

        # ---------------- per-batch stages (software-pipelined emission) ----
        # state keyed by (batch, rep parity) so rep r+1's front half can be
        # emitted while rep r's tail still reads its own tiles
        S = {}

        def stage_load(b, rep_i):
            k = (b, rep_i & 1)
            if rep_i == 0 or not load_x:
                if rep_i == 0:
                    S[k] = {"xt": xts[b], "condT": condTs[b]}
                else:
                    S[k] = {"xt": S[(b, (rep_i - 1) & 1)]["xt"],
                            "condT": S[(b, (rep_i - 1) & 1)]["condT"]}
            else:
                xt = xpool.tile([128, NCH, N], bf16, tag="xt", name=f"xtr{b}")
                nc.sync.dma_start(xt, x_d[b].rearrange("(a p) n -> p a n", p=128))
                cta = bpool.tile([128, 1], bf16, tag="cta", name=f"ctar{b}")
                nc.sync.dma_start(cta,
                                  cond_d[b, 0:128].rearrange("(p o) -> p o", o=1))
                ctb = bpool.tile([52, 1], bf16, tag="ctb", name=f"ctbr{b}")
                nc.sync.dma_start(ctb,
                                  cond_d[b, 128:180].rearrange("(p o) -> p o", o=1))
                S[k] = {"xt": xt, "condT": (cta, ctb)}

        def stage_stats_a(k):
            xt = S[k]["xt"]
            mv2 = bpool.tile([128, NCH, 2], f32, tag="mv2", name=f"mv2_{k[0]}")
            mv = bpool.tile([128, NCH, 2], f32, tag="mv", name=f"mv_{k[0]}")
            for ch in range(NCH):
                st = bpool.tile([128, 2, 6], f32, tag="st", name=f"st{k[0]}{ch}")
                nc.vector.bn_stats(st[:, 0, :], xt[:, ch, 0:512])
                nc.vector.bn_stats(st[:, 1, :], xt[:, ch, 512:1024])
                nc.vector.bn_aggr(mv[:, ch, :], st)
            msq = bpool.tile([128, NCH], f32, tag="msq", name=f"msq{k[0]}")
            nc.vector.tensor_mul(msq, mv[:, :, 0], mv[:, :, 0])
            nc.vector.tensor_copy(mv2[:, :, 0], mv[:, :, 0])
            nc.vector.tensor_add(mv2[:, :, 1], mv[:, :, 1], msq)
            S[k]["mv2"] = mv2

        def stage_stats_b(k):
            mv2 = S[k]["mv2"]
            gstat_ps = ps_sm.tile([8, NCH, 2], f32, tag="sm", name=f"gst{k[0]}")
            for ch in range(NCH):
                nc.tensor.matmul(gstat_ps[:, ch, :], ind128, mv2[:, ch, :],
                                 start=True, stop=True)
            gsb = bpool.tile([8, NCH, 2], f32, tag="gsb", name=f"gsb{k[0]}")
            nc.vector.tensor_copy(gsb, gstat_ps)
            msqg = bpool.tile([8, NCH], f32, tag="msqg", name=f"msqg{k[0]}")
            nc.vector.tensor_mul(msqg, gsb[:, :, 0], gsb[:, :, 0])
            varg = bpool.tile([8, NCH], f32, tag="varg", name=f"varg{k[0]}")
            nc.vector.tensor_sub(varg, gsb[:, :, 1], msqg)
            lnv = bpool.tile([8, NCH], f32, tag="lnv", name=f"lnv{k[0]}")
            nc.scalar.activation(lnv, varg, Act.Ln, bias=eps8[:, 0:1])
            rm = bpool.tile([8, 2, NCH], f32, tag="rm", name=f"rm{k[0]}")
            nc.scalar.activation(rm[:, 0, :], lnv, Act.Exp, scale=-0.5)
            nc.vector.tensor_mul(rm[:, 1, :], gsb[:, :, 0], rm[:, 0, :])
            rep_ps = ps_sm.tile([128, 2 * NCH], f32, tag="sm", name=f"rep{k[0]}")
            nc.tensor.matmul(rep_ps, indT8, rm.rearrange("g a c -> g (a c)"),
                             start=True, stop=True)
            rep3 = rep_ps.rearrange("p (a c) -> p a c", a=2)
            a_all = bpool.tile([128, NCH], bf16, tag="a_all", name=f"a_all{k[0]}")
            nc.vector.tensor_mul(a_all, wqg_pc, rep3[:, 0, :])
            wm_all = bpool.tile([128, NCH], f32, tag="wm_all", name=f"wm{k[0]}")
            nc.vector.tensor_mul(wm_all, wqg_pc, rep3[:, 1, :])
            S[k]["a_all"], S[k]["wm_all"] = a_all, wm_all

        def stage_smv_mm(k):
            a_all, wm_all, xt = S[k]["a_all"], S[k]["wm_all"], S[k]["xt"]
            acc = [ps_acc.tile([128, 512], f32, tag="acc", name=f"acc{k[0]}{h}")
                   for h in range(2)]
            wms_ps = ps_sm.tile([1, 1], f32, tag="sm", name=f"wms{k[0]}")
            for ch in range(NCH):
                for h in range(2):
                    nc.tensor.matmul(
                        acc[h][0:1, :], a_all[:, ch:ch + 1],
                        xt[:, ch, 512 * h:512 * (h + 1)],
                        start=(ch == 0), stop=(ch == NCH - 1),
                        skip_group_check=True)
                nc.tensor.matmul(wms_ps, wm_all[:, ch:ch + 1], ones_col,
                                 start=(ch == 0), stop=(ch == NCH - 1))
            S[k]["acc"], S[k]["wms_ps"] = acc, wms_ps

        def stage_s(k):
            acc, wms_ps = S[k]["acc"], S[k]["wms_ps"]
            constb = bpool.tile([1, 1], f32, tag="constb", name=f"cb{k[0]}")
            nc.vector.tensor_sub(constb, wqb_sb, wms_ps)
            s_sb = bpool.tile([1, N], bf16, tag="s_sb", name=f"s_sb{k[0]}")
            for h in range(2):
                nc.vector.tensor_scalar_add(s_sb[0:1, 512 * h:512 * (h + 1)],
                                            acc[h][0:1, :], constb[0:1, 0:1])
            S[k]["s_sb"] = s_sb

        def stage_kv(k):
            cta, ctb = S[k]["condT"]
            kv_ps = ps_sm.tile([128, 2 * NFC], f32, tag="sm", name=f"kv{k[0]}")
            for fc in range(NFC):
                nc.tensor.matmul(kv_ps[:, fc:fc + 1],
                                 wkTa[:, 128 * fc:128 * (fc + 1)], cta,
                                 start=True, stop=False, skip_group_check=True)
                nc.tensor.matmul(kv_ps[:, fc:fc + 1],
                                 wkTb[:, 128 * fc:128 * (fc + 1)], ctb,
                                 start=False, stop=True, skip_group_check=True)
            for fc in range(NFC):
                nc.tensor.matmul(kv_ps[:, NFC + fc:NFC + fc + 1],
                                 wvTa[:, 128 * fc:128 * (fc + 1)], cta,
                                 start=True, stop=False, skip_group_check=True)
                nc.tensor.matmul(kv_ps[:, NFC + fc:NFC + fc + 1],
                                 wvTb[:, 128 * fc:128 * (fc + 1)], ctb,
                                 start=False, stop=True, skip_group_check=True)
            kbs = bpool.tile([128, NFC], f32, tag="kbs", name=f"kbs{k[0]}")
            nc.vector.tensor_add(kbs, kv_ps[:, 0:NFC], bks_pc)
            vbp_b = bpool.tile([128, NFC], bf16, tag="vbp_b", name=f"vbp{k[0]}")
            nc.vector.tensor_add(vbp_b, kv_ps[:, NFC:2 * NFC], bv_pc)
            vbones = bpool.tile([128, NFC, 33], bf16, tag="vbones",
                                name=f"vo{k[0]}")
            nc.gpsimd.memset(vbones, 0.0)
            nc.gpsimd.tensor_copy(vbones[:, :, 0:1],
                                  vbp_b.rearrange("p (f o) -> p f o", o=1))
            nc.gpsimd.memset(vbones[:, :, 32:33], 1.0)
            S[k]["kbs"], S[k]["vbones"] = kbs, vbones

        def stage_expz(k, hooks=None):
            acc, s_sb = S[k]["acc"], S[k]["s_sb"]
            kbs, vbones = S[k]["kbs"], S[k]["vbones"]
            srep_ps = ps_srep.tile([128, N], f32, tag="srep", name=f"srep{k[0]}")
            for h in range(2):
                nc.tensor.matmul(srep_ps[:, 512 * h:512 * (h + 1)], ones_row_b,
                                 s_sb[0:1, 512 * h:512 * (h + 1)],
                                 start=True, stop=True)
            for fc in range(NFC):
                if hooks and fc in hooks:
                    hooks[fc]()
                e_sb = epool.tile([128, N], bf16, tag="e", name=f"e{k[0]}{fc}")
                nc.scalar.activation(e_sb, srep_ps, Act.Exp,
                                     scale=kbs[:, fc:fc + 1])
                for h in range(2):
                    nc.tensor.matmul(acc[h][0:33, :], vbones[:, fc, :],
                                     e_sb[:, 512 * h:512 * (h + 1)],
                                     start=(fc == 0), stop=(fc == NFC - 1),
                                     skip_group_check=True)

        def stage_w_compute(k):
            acc = S[k]["acc"]
            w_sb = bpool.tile([1, N], bf16, tag="w_sb", name=f"w_sb{k[0]}")
            for h in range(2):
                zr = bpool.tile([1, 512], f32, tag="zr", name=f"zr{k[0]}{h}")
                nc.vector.reciprocal(zr, acc[h][32:33, :])
                nc.vector.tensor_mul(w_sb[0:1, 512 * h:512 * (h + 1)],
                                     acc[h][0:1, :], zr)
            S[k]["w_sb"] = w_sb

        def stage_w_bcast(k, eng):
            w_sb = S[k]["w_sb"]
            wrep_sb = bpool.tile([128, N], bf16, tag="wrep_sb",
                                 name=f"wrep{k[0]}")
            wrep_ps = ps_sm.tile([128, 512], f32, tag="sm", name=f"wrep{k[0]}")
            for h in range(2):
                nc.tensor.matmul(wrep_ps, ones_row_b,
                                 w_sb[0:1, 512 * h:512 * (h + 1)],
                                 start=True, stop=True)
                if eng == "act":
                    nc.scalar.copy(wrep_sb[:, 512 * h:512 * (h + 1)], wrep_ps)
                else:
                    nc.vector.tensor_copy(wrep_sb[:, 512 * h:512 * (h + 1)],
                                          wrep_ps)
            S[k]["wrep_sb"] = wrep_sb

        def stage_yout(k):
            xt, wrep_sb = S[k]["xt"], S[k]["wrep_sb"]
            o_sb = opool.tile([128, NCH, N], bf16, tag="o", name=f"o{k[0]}")
            for ch in range(NCH):
                y_sb = ypool.tile([128, N], bf16, tag="y", name=f"y{k[0]}{ch}")
                nc.gpsimd.tensor_scalar(y_sb, wrep_sb, wors_pc[:, ch:ch + 1],
                                        bo_pc[:, ch:ch + 1], op0=Alu.mult,
                                        op1=Alu.add)
                nc.gpsimd.tensor_add(o_sb[:, ch, :], xt[:, ch, :], y_sb)
            if store_out:
                nc.sync.dma_start(
                    out_d[k[0]].rearrange("(a p) n -> p a n", p=128), o_sb)

        def front_half(b, rep_i):
            stage_load(b, rep_i)
            k = (b, rep_i & 1)
            stage_stats_a(k)
            stage_stats_b(k)
            stage_smv_mm(k)

        # rep 0 front half for batch 0 is not overlapped with anything
        front_half(0, 0)
        stage_s((0, 0))
        stage_kv((0, 0))
        for rep_i in range(reps):
            p = rep_i & 1
            k0, k1 = (0, p), (1, p)
            stage_expz(k0, hooks={
                1: lambda: (stage_load(1, rep_i), stage_stats_a(k1)),
                4: lambda: stage_stats_b(k1),
                6: lambda: stage_smv_mm(k1),
            } if rep_i == 0 else {
                4: lambda: stage_stats_b(k1),
                6: lambda: stage_smv_mm(k1),
            })
            stage_s(k1)
            stage_kv(k1)
            stage_w_compute(k0)
            np_ = (rep_i + 1) & 1
            if rep_i + 1 < reps:
                stage_expz(k1, hooks={
                    1: lambda: (stage_load(0, rep_i + 1),
                                stage_load(1, rep_i + 1),
                                stage_stats_a((0, np_))),
                    4: lambda: stage_stats_b((0, np_)),
                    5: lambda: stage_w_bcast(k0, "act"),
                    6: lambda: stage_smv_mm((0, np_)),
                })
            else:
                stage_expz(k1, hooks={5: lambda: stage_w_bcast(k0, "act")})
            stage_yout(k0)
            if rep_i + 1 < reps:
                stage_s((0, np_))
                stage_kv((0, np_))
            stage_w_compute(k1)
            stage_w_bcast(k1, "dve")
            stage_yout(k1)
            if rep_i + 1 < reps:
                stage_stats_a((1, np_))

    if legalize:
        _legalize_sync(nc, mybir)
    return nc


def _indicators():
    ind128 = np.zeros((128, 8), np.float32)
    indT8 = np.zeros((8, 128), np.float32)
    for g in range(8):
        ind128[16 * g:16 * g + 16, g] = 1.0 / 16.0
        indT8[g, 16 * g:16 * g + 16] = 1.0
    return ind128, indT8


def _host_prep(inputs):
    import ml_dtypes
    bf = ml_dtypes.bfloat16
    f = {k: np.ascontiguousarray(np.asarray(v, dtype=np.float32))
         for k, v in inputs.items()}
    x = f["x"].reshape(B, C, N).astype(bf)
    cond = f["condition"].astype(bf)
    colsum = f["wq"].sum(axis=0)                       # [C]
    wqg = (colsum * f["gamma"]).astype(np.float32)
    wqb = np.array([f["bq"].sum() + (colsum * f["beta"]).sum()], np.float32)
    wo_rs = f["wo"].sum(axis=1).astype(np.float32)     # [C]
    wkT = np.ascontiguousarray((f["wk"] * SCALE).T).astype(bf)   # [T,F]
    wvT = np.ascontiguousarray(f["wv"].T).astype(bf)
    bks = (f["bk"] * SCALE).astype(np.float32)
    ind128, indT8 = _indicators()
    common = {
        "wqg": wqg, "wqb": wqb, "wo_rs": wo_rs, "bo": f["bo"],
        "bks": bks, "bv": f["bv"], "wkT": wkT, "wvT": wvT,
        "ind128": ind128, "indT8": indT8,
    }
    return x, cond, common


def kernel(**inputs):
    from concourse.bass_utils import run_bass_kernel_spmd

    if "nc" not in _CACHE:
        _CACHE["nc"] = _build()
    nc = _CACHE["nc"]

    x, cond, common = _host_prep(inputs)
    in_maps = []
    for i in range(NCORES):
        in_maps.append({
            "x_sh": np.ascontiguousarray(x[BPC * i:BPC * (i + 1)]),
            "cond_sh": np.ascontiguousarray(cond[BPC * i:BPC * (i + 1)]),
            **common,
        })

    res = run_bass_kernel_spmd(nc, in_maps, core_ids=list(range(NCORES)))
    _CACHE["last_results"] = res
    out = np.concatenate([r["out"] for r in res.results], axis=0)
    return out.reshape(B, C, HW, HW).astype(np.float32)


# revision 36
# speedup vs baseline: 3.0094x; 1.2228x over previous
"""Trainium2 Bass kernel for nn_AttnBlock (B=16, C=512, H=W=32, T=180, G=32).

Math: the module broadcasts the text condition across channels, so k/v rows are
identical for every channel and the whole attention block collapses to rank-1:

  per batch b:
    group-norm stats over x[b]:   mu_g, rstd_g (32 groups of 16 ch x 1024 pix)
    wqg[c] = (sum_o wq[o,c]) * gamma[c];  a[c] = wqg[c]*rstd_{g(c)}
    s[n]   = sum_c a[c]*x[c,n]            (+ const_b, folded into the exp bias)
    kb[f]  = SCALE*(wk @ cond_b + bk) ;  vb[f] = wv @ cond_b + bv
    e[f,n] = exp(kb[f]*s[n] + kb[f]*const_b)
    w[n]   = (sum_f vb[f]*e[f,n]) / (sum_f e[f,n])
    out[c,n] = x[c,n] + wo_rowsum[c]*w[n] + bo[c]

Sharding: data-parallel over batch, 2 batches per core, 8 cores, no collectives.

v2: x/out/cond ride HBM as bf16 (halves DMA, kills the cast, 2x DVE modes);
k/v matvecs run on PE against host-transposed bf16 weights; exp reads the
s-broadcast straight from PSUM with const_b folded into a per-partition bias;
weight-derived scalars (wq colsum, wo rowsum, bq/beta folds) are host-folded.
PSUM: 4 accumulator banks (2/batch: s row @p0, softmax num @p0/den @p32 after
re-start) + 2 broadcast banks (srep/wrep, reused h0->h1) + 2 small banks.
"""
import numpy as np
from contextlib import ExitStack

B, C, HW, N, T = 16, 512, 32, 1024, 180
F = 1024                      # in_features == H*W
G = 32                        # groups; 16 channels per group
NCORES, BPC = 8, 2            # cores, batches per core
NCH = C // 128                # 4 channel chunks
NFC = F // 128                # 8 feature chunks
EPS = 1e-6
SCALE = float(C) ** -0.5

_CACHE = {}


def _legalize_sync(nc, mybir):
    """This walrus build accepts at most one sync-wait command per
    instruction; hoist extra waits onto preceding same-engine NOPs."""
    k = 0
    for fn in nc.m.functions:
        for blk in fn.blocks:
            new = []
            for ins in blk.instructions:
                si = ins.sync_info
                if si is not None and si.on_wait is not None and len(si.on_wait) > 1:
                    for w in list(si.on_wait[:-1]):
                        nop = mybir.InstNoOp(name=f"syncsplit-{k}", ins=[], outs=[])
                        k += 1
                        nop.engine = ins.engine
                        nop.sync_info = mybir.SyncInfo(on_wait=[w], on_update=[])
                        new.append(nop)
                    ins.sync_info = mybir.SyncInfo(
                        on_wait=[si.on_wait[-1]],
                        on_update=list(si.on_update or []))
                new.append(ins)
            blk.instructions[:] = new


def _build(reps=1, legalize=True, store_out=True, load_x=True, xbufs=4):
    import concourse.bass as bass
    import concourse.mybir as mybir
    import concourse.tile as tile

    f32 = mybir.dt.float32
    bf16 = mybir.dt.bfloat16
    Act = mybir.ActivationFunctionType
    Alu = mybir.AluOpType

    nc = bass.Bass()

    x_d = nc.dram_tensor("x_sh", [BPC, C, N], bf16, kind="ExternalInput")
    cond_d = nc.dram_tensor("cond_sh", [BPC, T], bf16, kind="ExternalInput")
    wqg_d = nc.dram_tensor("wqg", [C], f32, kind="ExternalInput")
    wors_d = nc.dram_tensor("wo_rs", [C], f32, kind="ExternalInput")
    bo_d = nc.dram_tensor("bo", [C], f32, kind="ExternalInput")
    bks_d = nc.dram_tensor("bks", [F], f32, kind="ExternalInput")
    bv_d = nc.dram_tensor("bv", [F], f32, kind="ExternalInput")
    wqb_d = nc.dram_tensor("wqb", [1], f32, kind="ExternalInput")
    wkT_d = nc.dram_tensor("wkT", [T, F], bf16, kind="ExternalInput")
    wvT_d = nc.dram_tensor("wvT", [T, F], bf16, kind="ExternalInput")
    ind128_d = nc.dram_tensor("ind128", [128, 8], f32, kind="ExternalInput")
    indT8_d = nc.dram_tensor("indT8", [8, 128], f32, kind="ExternalInput")
    out_d = nc.dram_tensor("out", [BPC, C, N], bf16, kind="ExternalOutput")

    with tile.TileContext(nc) as tc, ExitStack() as ctx:
        singles = ctx.enter_context(tc.tile_pool(name="singles", bufs=1))
        xpool = ctx.enter_context(tc.tile_pool(name="xpool", bufs=xbufs))
        ypool = ctx.enter_context(tc.tile_pool(name="ypool", bufs=4))
        opool = ctx.enter_context(tc.tile_pool(name="opool", bufs=3))
        bpool = ctx.enter_context(tc.tile_pool(name="bpool", bufs=2))
        ps_acc = ctx.enter_context(tc.tile_pool(name="ps_acc", bufs=4, space="PSUM"))
        ps_sm = ctx.enter_context(tc.tile_pool(name="ps_sm", bufs=4, space="PSUM"))

        # constants + ACT exp-table preload first (ACT ring is in-order)
        ones_col = singles.tile([128, 1], f32)
        nc.vector.memset(ones_col, 1.0)
        ones_row_b = singles.tile([1, 128], bf16)
        nc.vector.memset(ones_row_b, 1.0)
        ones_row_n = singles.tile([1, N], bf16)
        nc.vector.memset(ones_row_n, 1.0)
        factrow = singles.tile([1, 8], f32)
        nc.vector.memset(factrow, 1.0)
        nc.vector.memset(factrow[0:1, 2:3], 0.5)
        nc.vector.memset(factrow[0:1, 3:4], 1.0 / 6.0)
        nc.vector.memset(factrow[0:1, 6:7], 0.5)
        nc.vector.memset(factrow[0:1, 7:8], 1.0 / 6.0)
        eps8 = singles.tile([8, 1], f32)
        nc.vector.memset(eps8, EPS)
        tl = singles.tile([1, 1], f32)
        nc.scalar.activation(tl, eps8[0:1, 0:1], Act.Exp)

        # ---------------- prologue loads (ACT ring; idle during body) -------
        xts, condTs = [], []
        xt0 = xpool.tile([128, NCH, N], bf16, tag="xt", name="xt0")
        nc.sync.dma_start(xt0, x_d[0].rearrange("(a p) n -> p a n", p=128))
        xt1 = xpool.tile([128, NCH, N], bf16, tag="xt", name="xt1")
        nc.sync.dma_start(xt1, x_d[1].rearrange("(a p) n -> p a n", p=128))
        xts.append(xt0)
        xts.append(xt1)
        for b in range(BPC):
            cta = bpool.tile([128, 1], bf16, tag="cta", name=f"cta{b}")
            nc.sync.dma_start(cta, cond_d[b, 0:128].rearrange("(p o) -> p o", o=1))
            ctb = bpool.tile([52, 1], bf16, tag="ctb", name=f"ctb{b}")
            nc.sync.dma_start(ctb, cond_d[b, 128:180].rearrange("(p o) -> p o", o=1))
            condTs.append((cta, ctb))
        wkTa = singles.tile([128, F], bf16)
        nc.scalar.dma_start(wkTa, wkT_d[0:128, :])
        wkTb = singles.tile([52, F], bf16)
        nc.scalar.dma_start(wkTb, wkT_d[128:180, :])
        wvTa = singles.tile([128, F], bf16)
        nc.scalar.dma_start(wvTa, wvT_d[0:128, :])
        wvTb = singles.tile([52, F], bf16)
        nc.scalar.dma_start(wvTb, wvT_d[128:180, :])
        ind128 = singles.tile([128, 8], f32)
        nc.scalar.dma_start(ind128, ind128_d[:, :])
        indT8 = singles.tile([8, 128], f32)
        nc.scalar.dma_start(indT8, indT8_d[:, :])
        wqg_pc = singles.tile([128, NCH], f32)
        nc.scalar.dma_start(wqg_pc, wqg_d[:].rearrange("(a p) -> p a", p=128))
        wors_pc = singles.tile([128, NCH], f32)
        nc.scalar.dma_start(wors_pc, wors_d[:].rearrange("(a p) -> p a", p=128))
        bo_pc = singles.tile([128, NCH], f32)
        nc.scalar.dma_start(bo_pc, bo_d[:].rearrange("(a p) -> p a", p=128))
        bks_pc = singles.tile([128, NFC], f32)
        nc.scalar.dma_start(bks_pc, bks_d[:].rearrange("(a p) -> p a", p=128))
        bv_pc = singles.tile([128, NFC], f32)
        nc.scalar.dma_start(bv_pc, bv_d[:].rearrange("(a p) -> p a", p=128))
        wqb_sb = singles.tile([1, 1], f32)
        nc.scalar.dma_start(wqb_sb, wqb_d[:].rearrange("(p o) -> p o", o=1))

        # ---------------- per-batch stages (software-pipelined emission) ----
        # state keyed by (batch, rep parity) so rep r+1's front half can be
        # emitted while rep r's tail still reads its own tiles
        S = {}

        def stage_load(b, rep_i):
            k = (b, rep_i & 1)
            if rep_i == 0 or not load_x:
                if rep_i == 0:
                    S[k] = {"xt": xts[b], "condT": condTs[b]}
                else:
                    S[k] = {"xt": S[(b, (rep_i - 1) & 1)]["xt"],
                            "condT": S[(b, (rep_i - 1) & 1)]["condT"]}
            else:
                xt = xpool.tile([128, NCH, N], bf16, tag="xt", name=f"xtr{b}")
                nc.sync.dma_start(xt, x_d[b].rearrange("(a p) n -> p a n", p=128))
                cta = bpool.tile([128, 1], bf16, tag="cta", name=f"ctar{b}")
                nc.scalar.dma_start(cta,
                                    cond_d[b, 0:128].rearrange("(p o) -> p o", o=1))
                ctb = bpool.tile([52, 1], bf16, tag="ctb", name=f"ctbr{b}")
                nc.scalar.dma_start(ctb,
                                    cond_d[b, 128:180].rearrange("(p o) -> p o", o=1))
                S[k] = {"xt": xt, "condT": (cta, ctb)}

        def stage_stats_a(k):
            xt = S[k]["xt"]
            mv2 = bpool.tile([128, NCH, 2], f32, tag="mv2", name=f"mv2_{k[0]}")
            mv = bpool.tile([128, NCH, 2], f32, tag="mv", name=f"mv_{k[0]}")
            for ch in range(NCH):
                st = bpool.tile([128, 2, 6], f32, tag="st", name=f"st{k[0]}{ch}")
                nc.vector.bn_stats(st[:, 0, :], xt[:, ch, 0:512])
                nc.vector.bn_stats(st[:, 1, :], xt[:, ch, 512:1024])
                nc.vector.bn_aggr(mv[:, ch, :], st)
            msq = bpool.tile([128, NCH], f32, tag="msq", name=f"msq{k[0]}")
            nc.vector.tensor_mul(msq, mv[:, :, 0], mv[:, :, 0])
            nc.vector.tensor_copy(mv2[:, :, 0], mv[:, :, 0])
            nc.vector.tensor_add(mv2[:, :, 1], mv[:, :, 1], msq)
            S[k]["mv2"] = mv2

        def stage_stats_b(k):
            mv2 = S[k]["mv2"]
            gstat_ps = ps_sm.tile([8, NCH, 2], f32, tag="sm", name=f"gst{k[0]}")
            for ch in range(NCH):
                nc.tensor.matmul(gstat_ps[:, ch, :], ind128, mv2[:, ch, :],
                                 start=True, stop=True)
            gsb = bpool.tile([8, NCH, 2], f32, tag="gsb", name=f"gsb{k[0]}")
            nc.vector.tensor_copy(gsb, gstat_ps)
            msqg = bpool.tile([8, NCH], f32, tag="msqg", name=f"msqg{k[0]}")
            nc.vector.tensor_mul(msqg, gsb[:, :, 0], gsb[:, :, 0])
            varg = bpool.tile([8, NCH], f32, tag="varg", name=f"varg{k[0]}")
            nc.vector.tensor_sub(varg, gsb[:, :, 1], msqg)
            lnv = bpool.tile([8, NCH], f32, tag="lnv", name=f"lnv{k[0]}")
            nc.scalar.activation(lnv, varg, Act.Ln, bias=eps8[:, 0:1])
            rm = bpool.tile([8, 2, NCH], f32, tag="rm", name=f"rm{k[0]}")
            nc.scalar.activation(rm[:, 0, :], lnv, Act.Exp, scale=-0.5)
            nc.vector.tensor_mul(rm[:, 1, :], gsb[:, :, 0], rm[:, 0, :])
            rep_ps = ps_sm.tile([128, 2 * NCH], f32, tag="sm", name=f"rep{k[0]}")
            nc.tensor.matmul(rep_ps, indT8, rm.rearrange("g a c -> g (a c)"),
                             start=True, stop=True)
            rep3 = rep_ps.rearrange("p (a c) -> p a c", a=2)
            a_all = bpool.tile([128, NCH], bf16, tag="a_all", name=f"a_all{k[0]}")
            nc.vector.tensor_mul(a_all, wqg_pc, rep3[:, 0, :])
            wm_all = bpool.tile([128, NCH], f32, tag="wm_all", name=f"wm{k[0]}")
            nc.vector.tensor_mul(wm_all, wqg_pc, rep3[:, 1, :])
            S[k]["a_all"], S[k]["wm_all"] = a_all, wm_all

        def stage_smv_mm(k):
            a_all, wm_all, xt = S[k]["a_all"], S[k]["wm_all"], S[k]["xt"]
            acc = [ps_acc.tile([128, 512], f32, tag="acc", name=f"acc{k[0]}{h}")
                   for h in range(2)]
            wms_ps = ps_sm.tile([1, 1], f32, tag="sm", name=f"wms{k[0]}")
            for ch in range(NCH):
                for h in range(2):
                    nc.tensor.matmul(
                        acc[h][0:1, :], a_all[:, ch:ch + 1],
                        xt[:, ch, 512 * h:512 * (h + 1)],
                        start=(ch == 0), stop=(ch == NCH - 1),
                        skip_group_check=True)
                nc.tensor.matmul(wms_ps, wm_all[:, ch:ch + 1], ones_col,
                                 start=(ch == 0), stop=(ch == NCH - 1))
            S[k]["acc"], S[k]["wms_ps"] = acc, wms_ps

        def stage_s(k):
            """power rows (each [1,N] at partition 0): s', s'^2, s'^3."""
            acc, wms_ps = S[k]["acc"], S[k]["wms_ps"]
            constb = bpool.tile([1, 1], f32, tag="constb", name=f"cb{k[0]}")
            nc.vector.tensor_sub(constb, wqb_sb, wms_ps)
            s1 = bpool.tile([1, N], bf16, tag="s1", name=f"s1_{k[0]}")
            for h in range(2):
                nc.scalar.activation(s1[0:1, 512 * h:512 * (h + 1)],
                                     acc[h][0:1, :], Act.Identity,
                                     bias=constb[0:1, 0:1])
            s2 = bpool.tile([1, N], bf16, tag="s2", name=f"s2_{k[0]}")
            nc.scalar.activation(s2, s1, Act.Square)
            s3 = bpool.tile([1, N], bf16, tag="s3", name=f"s3_{k[0]}")
            nc.gpsimd.tensor_mul(s3, s2, s1)
            S[k]["srows"] = (ones_row_n, s1, s2, s3)

        def stage_kv(k):
            cta, ctb = S[k]["condT"]
            kv_ps = ps_sm.tile([128, 2 * NFC], f32, tag="sm", name=f"kv{k[0]}")
            for fc in range(NFC):
                nc.tensor.matmul(kv_ps[:, fc:fc + 1],
                                 wkTa[:, 128 * fc:128 * (fc + 1)], cta,
                                 start=True, stop=False, skip_group_check=True)
                nc.tensor.matmul(kv_ps[:, fc:fc + 1],
                                 wkTb[:, 128 * fc:128 * (fc + 1)], ctb,
                                 start=False, stop=True, skip_group_check=True)
            for fc in range(NFC):
                nc.tensor.matmul(kv_ps[:, NFC + fc:NFC + fc + 1],
                                 wvTa[:, 128 * fc:128 * (fc + 1)], cta,
                                 start=True, stop=False, skip_group_check=True)
                nc.tensor.matmul(kv_ps[:, NFC + fc:NFC + fc + 1],
                                 wvTb[:, 128 * fc:128 * (fc + 1)], ctb,
                                 start=False, stop=True, skip_group_check=True)
            S[k]["kv_ps"] = kv_ps

        def stage_kv_post(k):
            kv_ps = S[k]["kv_ps"]
            kbs = bpool.tile([128, NFC], f32, tag="kbs", name=f"kbs{k[0]}")
            nc.vector.tensor_add(kbs, kv_ps[:, 0:NFC], bks_pc)
            vbp = bpool.tile([128, NFC], f32, tag="vbp", name=f"vbp{k[0]}")
            nc.vector.tensor_add(vbp, kv_ps[:, NFC:2 * NFC], bv_pc)
            S[k]["kbs"], S[k]["vbp"] = kbs, vbp

        def stage_coef_pool(k):
            kbs, vbp = S[k]["kbs"], S[k]["vbp"]
            kpow = bpool.tile([128, NFC, 4], f32, tag="kpow", name=f"kp{k[0]}")
            nc.vector.memset(kpow[:, :, 0:1], 1.0)
            nc.vector.tensor_copy(kpow[:, :, 1:2],
                                  kbs.rearrange("p (f o) -> p f o", o=1))
            nc.vector.tensor_mul(kpow[:, :, 2], kbs, kbs)
            nc.vector.tensor_mul(kpow[:, :, 3], kpow[:, :, 2], kbs)
            kpown = bpool.tile([128, NFC, 4], f32, tag="kpown", name=f"kn{k[0]}")
            vb_b4 = bass.AP(tensor=vbp.tensor, offset=vbp.offset,
                            ap=[list(vbp.ap[0]), list(vbp.ap[1]), [0, 4]])
            nc.vector.tensor_mul(kpown, kpow, vb_b4)
            S[k]["kpow"], S[k]["kpown"] = kpow, kpown

        def stage_coef_q(k):
            """coefpair layout: [pn_0..pn_3 | pd_0..pd_3] (1/j! via factrow)"""
            kpow, kpown = S[k]["kpow"], S[k]["kpown"]
            q_ps = ps_sm.tile([1, 2, 4 * NFC], f32, tag="sm", name=f"q{k[0]}")
            nc.tensor.matmul(q_ps[:, 0, :], ones_col,
                             kpown.rearrange("p f j -> p (f j)"),
                             start=True, stop=True, skip_group_check=True)
            nc.tensor.matmul(q_ps[:, 1, :], ones_col,
                             kpow.rearrange("p f j -> p (f j)"),
                             start=True, stop=True, skip_group_check=True)
            # q layout: [T_0..T_3 | U_0..U_3] after f-reduce and 1/j! factors;
            # then the cubic Taylor of w = T(s)/U(s) itself:
            #   c0 = T0/U0, c1 = (T1-c0 U1)/U0, c2 = (T2-c0 U2-c1 U1)/U0,
            #   c3 = (T3-c0 U3-c1 U2-c2 U1)/U0
            coefraw = bpool.tile([1, 8], f32, tag="coefraw", name=f"cr{k[0]}")
            for c in range(2):
                qv = q_ps[:, c, :].rearrange("p (f j) -> p j f", f=NFC)
                nc.vector.tensor_reduce(coefraw[0:1, 4 * c:4 * (c + 1)],
                                        qv, axis=mybir.AxisListType.X,
                                        op=Alu.add)
            tu = bpool.tile([1, 8], f32, tag="tu", name=f"tu{k[0]}")
            nc.vector.tensor_mul(tu, coefraw, factrow)
            cw = bpool.tile([1, 4], f32, tag="cw", name=f"cw{k[0]}")
            sc = bpool.tile([1, 8], f32, tag="scr", name=f"scr{k[0]}")
            r0 = sc[0:1, 0:1]
            with nc.allow_low_precision(reason="scalar chain; 2e-2 budget"):
                nc.vector.reciprocal(r0, tu[0:1, 4:5])
                nc.vector.tensor_mul(cw[0:1, 0:1], tu[0:1, 0:1], r0)
                for j in range(1, 4):
                    acc_s = sc[0:1, j:j + 1]
                    nc.vector.tensor_mul(acc_s, cw[0:1, 0:1],
                                         tu[0:1, 4 + j:5 + j])
                    for i in range(1, j):
                        t_s = sc[0:1, 4 + i:5 + i]
                        nc.vector.tensor_mul(t_s, cw[0:1, i:i + 1],
                                             tu[0:1, 4 + j - i:5 + j - i])
                        nc.vector.tensor_add(acc_s, acc_s, t_s)
                    nc.vector.tensor_sub(acc_s, tu[0:1, j:j + 1], acc_s)
                    nc.vector.tensor_mul(cw[0:1, j:j + 1], acc_s, r0)
                cvec = bpool.tile([1, 4], bf16, tag="cvec", name=f"cv{k[0]}")
                nc.vector.tensor_copy(cvec, cw)
            S[k]["cvec"] = cvec

        def stage_eval(k):
            srows, cvec = S[k]["srows"], S[k]["cvec"]
            wps = [ps_sm.tile([1, 512], f32, tag="sm", name=f"wp{k[0]}{h}")
                   for h in range(2)]
            for h in range(2):
                for j in range(4):
                    nc.tensor.matmul(wps[h], cvec[0:1, j:j + 1],
                                     srows[j][0:1, 512 * h:512 * (h + 1)],
                                     start=(j == 0), stop=(j == 3),
                                     skip_group_check=True)
            S[k]["wps"] = wps

        def stage_w_bcast(k, eng):
            wps = S[k]["wps"]
            w_sb = bpool.tile([1, N], bf16, tag="w_sb", name=f"w_sb{k[0]}")
            wrep_sb = bpool.tile([128, N], bf16, tag="wrep_sb",
                                 name=f"wrep{k[0]}")
            wrep_ps = ps_sm.tile([128, 512], f32, tag="sm", name=f"wrep{k[0]}")
            with nc.allow_low_precision(reason="w bf16; 2e-2 budget"):
                for h in range(2):
                    hs = slice(512 * h, 512 * (h + 1))
                    if eng == "act":
                        nc.scalar.copy(w_sb[0:1, hs], wps[h])
                    else:
                        nc.vector.tensor_copy(w_sb[0:1, hs], wps[h])
                    nc.tensor.matmul(wrep_ps, ones_row_b, w_sb[0:1, hs],
                                     start=True, stop=True)
                    if eng == "act":
                        nc.scalar.copy(wrep_sb[:, hs], wrep_ps)
                    else:
                        nc.vector.tensor_copy(wrep_sb[:, hs], wrep_ps)
            S[k]["wrep_sb"] = wrep_sb

        def stage_yout(k):
            xt, wrep_sb = S[k]["xt"], S[k]["wrep_sb"]
            o_sb = opool.tile([128, NCH, N], bf16, tag="o", name=f"o{k[0]}")
            for ch in range(NCH):
                y_sb = ypool.tile([128, N], bf16, tag="y", name=f"y{k[0]}{ch}")
                nc.gpsimd.tensor_scalar(y_sb, wrep_sb, wors_pc[:, ch:ch + 1],
                                        bo_pc[:, ch:ch + 1], op0=Alu.mult,
                                        op1=Alu.add)
                nc.gpsimd.tensor_add(o_sb[:, ch, :], xt[:, ch, :], y_sb)
            if store_out:
                nc.sync.dma_start(
                    out_d[k[0]].rearrange("(a p) n -> p a n", p=128), o_sb)

        stage_load(0, 0)
        stage_load(1, 0)
        for rep_i in range(reps):
            p = rep_i & 1
            k0, k1 = (0, p), (1, p)
            if rep_i + 1 < reps:
                stage_load(0, rep_i + 1)
                stage_load(1, rep_i + 1)
            stage_stats_a(k0)
            stage_stats_b(k0)
            stage_stats_a(k1)
            stage_smv_mm(k0)
            stage_s(k0)
            stage_kv(k0)
            stage_kv_post(k0)
            stage_coef_pool(k0)
            stage_coef_q(k0)
            stage_stats_b(k1)
            stage_eval(k0)
            stage_w_bcast(k0, "act")
            stage_smv_mm(k1)
            stage_s(k1)
            stage_kv(k1)
            stage_kv_post(k1)
            stage_coef_pool(k1)
            stage_coef_q(k1)
            stage_yout(k0)
            stage_eval(k1)
            stage_w_bcast(k1, "dve")
            stage_yout(k1)

    if legalize:
        _legalize_sync(nc, mybir)
    return nc


def _indicators():
    ind128 = np.zeros((128, 8), np.float32)
    indT8 = np.zeros((8, 128), np.float32)
    for g in range(8):
        ind128[16 * g:16 * g + 16, g] = 1.0 / 16.0
        indT8[g, 16 * g:16 * g + 16] = 1.0
    return ind128, indT8


def _host_prep(inputs):
    import ml_dtypes
    bf = ml_dtypes.bfloat16
    f = {k: np.ascontiguousarray(np.asarray(v, dtype=np.float32))
         for k, v in inputs.items()}
    x = f["x"].reshape(B, C, N).astype(bf)
    cond = f["condition"].astype(bf)
    colsum = f["wq"].sum(axis=0)                       # [C]
    wqg = (colsum * f["gamma"]).astype(np.float32)
    wqb = np.array([f["bq"].sum() + (colsum * f["beta"]).sum()], np.float32)
    wo_rs = f["wo"].sum(axis=1).astype(np.float32)     # [C]
    wkT = np.ascontiguousarray((f["wk"] * SCALE).T).astype(bf)   # [T,F]
    wvT = np.ascontiguousarray(f["wv"].T).astype(bf)
    bks = (f["bk"] * SCALE).astype(np.float32)
    ind128, indT8 = _indicators()
    common = {
        "wqg": wqg, "wqb": wqb, "wo_rs": wo_rs, "bo": f["bo"],
        "bks": bks, "bv": f["bv"], "wkT": wkT, "wvT": wvT,
        "ind128": ind128, "indT8": indT8,
    }
    return x, cond, common


def kernel(**inputs):
    from concourse.bass_utils import run_bass_kernel_spmd

    if "nc" not in _CACHE:
        _CACHE["nc"] = _build()
    nc = _CACHE["nc"]

    x, cond, common = _host_prep(inputs)
    in_maps = []
    for i in range(NCORES):
        in_maps.append({
            "x_sh": np.ascontiguousarray(x[BPC * i:BPC * (i + 1)]),
            "cond_sh": np.ascontiguousarray(cond[BPC * i:BPC * (i + 1)]),
            **common,
        })

    res = run_bass_kernel_spmd(nc, in_maps, core_ids=list(range(NCORES)))
    _CACHE["last_results"] = res
    out = np.concatenate([r["out"] for r in res.results], axis=0)
    return out.reshape(B, C, HW, HW).astype(np.float32)
